# revision 4
# baseline (speedup 1.0000x reference)
"""GCN encoder kernel for 8 Trainium2 NeuronCores (v2).

Math: out = A_hat @ (x @ W2) + b_out with A_hat = D^-1/2 (A+I) D^-1/2,
W2 = W_gc @ W_fc, b_out = b_gc @ W_fc + b_fc. The dinv factors are folded
into the gather table (g' = (x @ W2) * dinv) so per-edge messages need no
norm multiply, and the self-loop term becomes an identity matmul of the
core-resident g' tile: out_t = dinv_t * (acc_t + g'_t) + b_out.

Distribution (dst-partitioned 1D graph parallel):
  - phase 1: each core computes g'_c = (x_c @ W2) * dinv_c from a
    host-transposed bf16 x_c^T (no PE transposes), keeps a bf16 copy of
    its 98 tiles resident in SBUF, DMAs fp32 to DRAM,
  - AllGather g' (fp32, 25.7 MB) so each core holds the full table,
  - phase 2: edges (no self-loops) bucketed by (dst tile, src group);
    14 supers x 7 tiles, 4 src groups of 25088 rows (int16-indexable);
    per (super, group) window one SWDGE dma_gather call (round-robin
    over the 4 queues), DVE casts the window to bf16 and builds per-chunk
    one-hots (iota == dst), PE segment-sums via PSUM-accumulated matmuls,
  - flush: identity matmul adds g'_t (self-loop), one DVE
    scalar_tensor_tensor applies dinv_t and bias, DMA to a partition-major
    output the host transposes back.
"""
import numpy as np
import ml_dtypes
from contextlib import ExitStack

N_NODES = 100000
IN_FEAT = 256
OUT_FEAT = 64
NCORES = 8
SHARD = N_NODES // NCORES          # 12500
NTILES = 98
PADSHARD = NTILES * 128            # 12544
GTAB_ROWS = NCORES * PADSHARD      # 100352
NGROUPS = 4
GROUP_ROWS = GTAB_ROWS // NGROUPS  # 25088 (int16-addressable)
SUPER_SIZES = [7] * 14             # 14 supers x 7 tiles
DST_SENTINEL = 255.0


def _preprocess(x, edge_index, W_gc, b_gc, W_fc, b_fc):
    x = np.asarray(x, np.float32)
    W2 = (np.asarray(W_gc, np.float64) @ np.asarray(W_fc, np.float64)).astype(np.float32)
    b_out = (np.asarray(b_gc, np.float64) @ np.asarray(W_fc, np.float64)
             + np.asarray(b_fc, np.float64)).astype(np.float32)

    src = np.asarray(edge_index[0], np.int64)
    dst = np.asarray(edge_index[1], np.int64)
    deg = np.bincount(dst, minlength=N_NODES).astype(np.float64) + 1.0
    dinv = (1.0 / np.sqrt(deg)).astype(np.float32)

    gsrc = (src // SHARD) * PADSHARD + (src % SHARD)
    grp = gsrc // GROUP_ROWS
    lidx = (gsrc % GROUP_ROWS).astype(np.int16)
    core = dst // SHARD
    dloc = dst % SHARD
    tile = dloc // 128
    dst_local = (dloc % 128).astype(np.float32)

    key = ((core * NTILES + tile) * NGROUPS + grp).astype(np.int64)
    order = np.argsort(key, kind="stable")
    key_s = key[order]
    lidx_s = lidx[order]
    dstl_s = dst_local[order]
    counts = np.bincount(key_s, minlength=NCORES * NTILES * NGROUPS)
    counts = counts.reshape(NCORES, NTILES, NGROUPS)
    starts = np.zeros_like(counts)
    flat = counts.reshape(NCORES, -1)
    starts.reshape(NCORES, -1)[:, 1:] = np.cumsum(flat, axis=1)[:, :-1]
    starts += np.concatenate([[0], np.cumsum(flat.sum(axis=1))[:-1]]).reshape(-1, 1, 1)

    chunks_tg = -(-counts.max(axis=0) // 128)              # [NTILES, NGROUPS]

    supers = []
    t0 = 0
    for ssz in SUPER_SIZES:
        supers.append(list(range(t0, t0 + ssz)))
        t0 += ssz

    # slot layout: super-major, then group, then tile-in-super
    slot_off = np.zeros((NTILES, NGROUPS), np.int64)
    windows = []                       # per (super, group): (w0, nchunks)
    super_rng = []                     # per super: (slot0, slot1)
    pos = 0
    for tiles in supers:
        s0 = pos
        for g in range(NGROUPS):
            w0 = pos
            for t in tiles:
                slot_off[t, g] = pos
                pos += int(chunks_tg[t, g]) * 128
            windows.append((w0, (pos - w0) // 128))
        super_rng.append((s0, pos))
    S_total = pos
    C_total = S_total // 128

    idx_all = np.zeros((NCORES, S_total), np.int16)
    dst_all = np.full((NCORES, S_total), DST_SENTINEL, np.float32)
    for c in range(NCORES):
        for t in range(NTILES):
            for g in range(NGROUPS):
                n = int(counts[c, t, g])
                if n == 0:
                    continue
                s0 = int(starts[c, t, g])
                o = int(slot_off[t, g])
                idx_all[c, o:o + n] = lidx_s[s0:s0 + n]
                dst_all[c, o:o + n] = dstl_s[s0:s0 + n]

    idx_dev = np.ascontiguousarray(
        np.tile(idx_all.reshape(NCORES, -1, 16).transpose(0, 2, 1), (1, 8, 1)))
    dst_dev = np.ascontiguousarray(
        dst_all.reshape(NCORES, C_total, 128).transpose(0, 2, 1)).astype(ml_dtypes.bfloat16)

    x_pad = np.zeros((NCORES, PADSHARD, IN_FEAT), np.float32)
    x_pad[:, :SHARD] = x.reshape(NCORES, SHARD, IN_FEAT)
    # xT_dev[c, p, h, n] = x_pad[c, n, h*128+p]
    xT_dev = np.ascontiguousarray(
        x_pad.transpose(0, 2, 1).reshape(NCORES, 2, 128, PADSHARD)
        .transpose(0, 2, 1, 3)).astype(ml_dtypes.bfloat16)

    dinv_pad = np.zeros((NCORES, PADSHARD), np.float32)
    dinv_pad[:, :SHARD] = dinv.reshape(NCORES, SHARD)
    dinv_dev = np.ascontiguousarray(
        dinv_pad.reshape(NCORES, NTILES, 128).transpose(0, 2, 1))   # [C,128,98]

    iota_np = np.tile(np.arange(128, dtype=np.float32)[None, :], (128, 1)).astype(ml_dtypes.bfloat16)
    identb_np = np.eye(128, dtype=np.float32).astype(ml_dtypes.bfloat16)
    bias_np = np.tile(b_out[None, :], (128, 1)).astype(np.float32)
    W2_np = W2.astype(ml_dtypes.bfloat16)

    meta = dict(chunks_tg=chunks_tg, supers=supers, windows=windows,
                slot_off=slot_off, S_total=S_total, C_total=C_total,
                super_rng=super_rng)
    per_core = dict(xT=xT_dev, idx=idx_dev, dstv=dst_dev, dinv=dinv_dev)
    consts = dict(W2=W2_np, iota=iota_np, identb=identb_np, bias=bias_np)
    return meta, per_core, consts


def _build(meta):
    import concourse.bass as bass
    import concourse.tile as tile
    from concourse import bacc, mybir

    chunks_tg = meta["chunks_tg"]
    supers = meta["supers"]
    windows = meta["windows"]
    slot_off = meta["slot_off"]
    S_total = meta["S_total"]
    C_total = meta["C_total"]
    super_rng = meta["super_rng"]

    nc = bacc.Bacc("TRN2", target_bir_lowering=False, debug=False,
                   num_devices=NCORES, num_swdge_queues=4)
    f32, bf16, i16 = mybir.dt.float32, mybir.dt.bfloat16, mybir.dt.int16
    mul_op, add_op = mybir.AluOpType.mult, mybir.AluOpType.add

    xT_ap = nc.dram_tensor("xT_in", [128, 2, PADSHARD], bf16, kind="ExternalInput").ap()
    idx_ap = nc.dram_tensor("idx_in", [128, S_total // 16], i16, kind="ExternalInput").ap()
    dst_ap = nc.dram_tensor("dst_in", [128, C_total], bf16, kind="ExternalInput").ap()
    dinv_ap = nc.dram_tensor("dinv_in", [128, NTILES], f32, kind="ExternalInput").ap()
    W2_ap = nc.dram_tensor("w2_in", [IN_FEAT, OUT_FEAT], bf16, kind="ExternalInput").ap()
    iota_ap = nc.dram_tensor("iota_in", [128, 128], bf16, kind="ExternalInput").ap()
    identb_ap = nc.dram_tensor("identb_in", [128, 128], bf16, kind="ExternalInput").ap()
    bias_ap = nc.dram_tensor("bias_in", [128, OUT_FEAT], f32, kind="ExternalInput").ap()
    out_ap = nc.dram_tensor("y_out", [128, NTILES, OUT_FEAT], f32, kind="ExternalOutput").ap()

    with tile.TileContext(nc) as tc, ExitStack() as ctx:
        dram = ctx.enter_context(tc.tile_pool(name="dram", bufs=1, space="DRAM"))
        g_c = dram.tile([PADSHARD, OUT_FEAT], f32)
        g_full = dram.tile([GTAB_ROWS, OUT_FEAT], f32)

        cpool = ctx.enter_context(tc.tile_pool(name="consts", bufs=1))
        iota_t = cpool.tile([128, 128], bf16)
        nc.sync.dma_start(iota_t[:], iota_ap[:])
        identb_t = cpool.tile([128, 128], bf16)
        nc.sync.dma_start(identb_t[:], identb_ap[:])
        bias_t = cpool.tile([128, OUT_FEAT], f32)
        nc.sync.dma_start(bias_t[:], bias_ap[:])
        dinv_t = cpool.tile([128, NTILES], f32)
        nc.sync.dma_start(dinv_t[:], dinv_ap[:])
        dstv_t = cpool.tile([128, C_total], bf16)
        nc.sync.dma_start(dstv_t[:], dst_ap[:])
        w2_t = cpool.tile([128, 2, OUT_FEAT], bf16)
        nc.sync.dma_start(w2_t[:], W2_ap.rearrange("(k p) f -> p k f", p=128))
        gres = cpool.tile([128, NTILES, OUT_FEAT], bf16)

        # ---- phase 1: g'_c = (x_c @ W2) * dinv_c ----
        with tc.tile_pool(name="ph1", bufs=3) as ph1, \
             tc.tile_pool(name="ph1c", bufs=1) as ph1c, \
             tc.tile_pool(name="ph1ps", bufs=2, space="PSUM") as ph1ps:
            xT_t = ph1c.tile([128, 2, PADSHARD], bf16)
            nc.sync.dma_start(xT_t[:], xT_ap[:])
            for nt in range(NTILES):
                gp = ph1ps.tile([128, OUT_FEAT], f32, tag="gp")
                for h in range(2):
                    nc.tensor.matmul(gp[:], xT_t[:, h, nt * 128:(nt + 1) * 128],
                                     w2_t[:, h, :], start=(h == 0), stop=(h == 1))
                gs = ph1.tile([128, OUT_FEAT], f32, tag="gs")
                nc.scalar.mul(gs[:], gp[:], dinv_t[:, nt:nt + 1])
                nc.vector.tensor_copy(gres[:, nt, :], gs[:])
                nc.sync.dma_start(g_c[nt * 128:(nt + 1) * 128, :], gs[:])

        # ---- allgather g' ----
        nc.gpsimd.collective_compute(
            "AllGather", mybir.AluOpType.bypass,
            ins=[g_c.opt()], outs=[g_full.opt()],
            replica_groups=[list(range(NCORES))],
        )

        # ---- phase 2 ----
        idxp = ctx.enter_context(tc.tile_pool(name="idxp", bufs=2))
        p2 = ctx.enter_context(tc.tile_pool(name="p2", bufs=2))
        ohp = ctx.enter_context(tc.tile_pool(name="ohp", bufs=6))
        psum2 = ctx.enter_context(tc.tile_pool(name="ps2", bufs=1, space="PSUM"))
        outp = ctx.enter_context(tc.tile_pool(name="outp", bufs=2))

        win_i = 0
        qrr = 0
        for si, tiles in enumerate(supers):
            s0, s1 = super_rng[si]
            idx_t = idxp.tile([128, (s1 - s0) // 16], i16, tag="idx")
            nc.sync.dma_start(idx_t[:], idx_ap[:, s0 // 16:s1 // 16])

            accs = {}
            started = set()
            for ti in range(len(tiles)):
                accs[ti] = psum2.tile([128, OUT_FEAT], f32, tag=f"acc{ti}",
                                      name=f"acc_{si}_{ti}")

            for g in range(NGROUPS):
                w0, wch = windows[win_i]
                win_i += 1
                if wch == 0:
                    continue
                nsl = wch * 128
                msg32 = p2.tile([128, wch, OUT_FEAT], f32, tag=f"m32_{g}", bufs=2)
                nc.gpsimd.dma_gather(
                    msg32[:], g_full[g * GROUP_ROWS:(g + 1) * GROUP_ROWS, :],
                    idx_t[:, (w0 - s0) // 16:(w0 - s0 + nsl) // 16],
                    nsl, nsl, OUT_FEAT, single_packet=False, queue_num=qrr % 4,
                )
                qrr += 1
                msg16 = p2.tile([128, wch, OUT_FEAT], bf16, tag=f"m16_{g}", bufs=2)
                nc.vector.tensor_copy(msg16[:], msg32[:])
                for ti, t in enumerate(tiles):
                    ntch = int(chunks_tg[t, g])
                    if ntch == 0:
                        continue
                    c0 = (slot_off[t, g] - w0) // 128
                    gc0 = slot_off[t, g] // 128
                    for j in range(ntch):
                        oh = ohp.tile([128, 128], bf16, tag="oh", bufs=6)
                        nc.vector.tensor_tensor(
                            out=oh[:], in0=iota_t[:],
                            in1=dstv_t[:, gc0 + j:gc0 + j + 1].broadcast_to([128, 128]),
                            op=mybir.AluOpType.is_equal)
                        nc.tensor.matmul(accs[ti][:], oh[:], msg16[:, c0 + j, :],
                                         start=(ti not in started), stop=False)
                        started.add(ti)

            stg = outp.tile([128, len(tiles), OUT_FEAT], f32, tag="stg")
            for ti, t in enumerate(tiles):
                nc.tensor.matmul(accs[ti][:], identb_t[:], gres[:, t, :],
                                 start=(ti not in started), stop=True)
                nc.vector.scalar_tensor_tensor(
                    out=stg[:, ti, :], in0=accs[ti][:],
                    scalar=dinv_t[:, t:t + 1], in1=bias_t[:],
                    op0=mul_op, op1=add_op)
            nc.sync.dma_start(out_ap[:, tiles[0]:tiles[0] + len(tiles), :], stg[:])

    nc.compile()
    return nc


_CACHED = {}


def kernel(x, edge_index, W_gc, b_gc, W_fc, b_fc):
    from concourse import bass_utils

    meta, per_core, consts = _preprocess(x, edge_index, W_gc, b_gc, W_fc, b_fc)
    cache_key = (meta["S_total"], meta["C_total"],
                 tuple(map(tuple, meta["chunks_tg"])))
    if cache_key in _CACHED:
        nc = _CACHED[cache_key]
    else:
        nc = _build(meta)
        _CACHED.clear()
        _CACHED[cache_key] = nc

    in_maps = []
    for c in range(NCORES):
        in_maps.append({
            "xT_in": per_core["xT"][c],
            "idx_in": per_core["idx"][c],
            "dst_in": per_core["dstv"][c],
            "dinv_in": per_core["dinv"][c],
            "w2_in": consts["W2"],
            "iota_in": consts["iota"],
            "identb_in": consts["identb"],
            "bias_in": consts["bias"],
        })
    res = bass_utils.run_bass_kernel_spmd(nc, in_maps, core_ids=list(range(NCORES)))
    out = np.empty((N_NODES, OUT_FEAT), np.float32)
    for c in range(NCORES):
        oc = res.results[c]["y_out"]
        out[c * SHARD:(c + 1) * SHARD] = (
            oc.transpose(1, 0, 2).reshape(PADSHARD, OUT_FEAT)[:SHARD])
    return out


# revision 5
# speedup vs baseline: 1.8914x; 1.8914x over previous
"""GCN encoder kernel for 8 Trainium2 NeuronCores (v2).

Math: out = A_hat @ (x @ W2) + b_out with A_hat = D^-1/2 (A+I) D^-1/2,
W2 = W_gc @ W_fc, b_out = b_gc @ W_fc + b_fc. The dinv factors are folded
into the gather table (g' = (x @ W2) * dinv) so per-edge messages need no
norm multiply, and the self-loop term becomes an identity matmul of the
core-resident g' tile: out_t = dinv_t * (acc_t + g'_t) + b_out.

Distribution (dst-partitioned 1D graph parallel):
  - phase 1: each core computes g'_c = (x_c @ W2) * dinv_c from a
    host-transposed bf16 x_c^T (no PE transposes), keeps a bf16 copy of
    its 98 tiles resident in SBUF, DMAs fp32 to DRAM,
  - AllGather g' (fp32, 25.7 MB) so each core holds the full table,
  - phase 2: edges (no self-loops) bucketed by (dst tile, src group);
    14 supers x 7 tiles, 4 src groups of 25088 rows (int16-indexable);
    per (super, group) window one SWDGE dma_gather call (round-robin
    over the 4 queues), DVE casts the window to bf16 and builds per-chunk
    one-hots (iota == dst), PE segment-sums via PSUM-accumulated matmuls,
  - flush: identity matmul adds g'_t (self-loop), one DVE
    scalar_tensor_tensor applies dinv_t and bias, DMA to a partition-major
    output the host transposes back.
"""
import numpy as np
import ml_dtypes
from contextlib import ExitStack

N_NODES = 100000
IN_FEAT = 256
OUT_FEAT = 64
NCORES = 8
SHARD = N_NODES // NCORES          # 12500
NTILES = 98
PADSHARD = NTILES * 128            # 12544
GTAB_ROWS = NCORES * PADSHARD      # 100352
NGROUPS = 4
GROUP_ROWS = GTAB_ROWS // NGROUPS  # 25088 (int16-addressable)
SUPER_SIZES = [7] * 14             # 14 supers x 7 tiles
DST_SENTINEL = 255.0


def _preprocess(x, edge_index, W_gc, b_gc, W_fc, b_fc):
    x = np.asarray(x, np.float32)
    W2 = (np.asarray(W_gc, np.float64) @ np.asarray(W_fc, np.float64)).astype(np.float32)
    b_out = (np.asarray(b_gc, np.float64) @ np.asarray(W_fc, np.float64)
             + np.asarray(b_fc, np.float64)).astype(np.float32)

    src = np.asarray(edge_index[0], np.int64)
    dst = np.asarray(edge_index[1], np.int64)
    deg = np.bincount(dst, minlength=N_NODES).astype(np.float64) + 1.0
    dinv = (1.0 / np.sqrt(deg)).astype(np.float32)

    gsrc = (src // SHARD) * PADSHARD + (src % SHARD)
    grp = gsrc // GROUP_ROWS
    lidx = (gsrc % GROUP_ROWS).astype(np.int16)
    core = dst // SHARD
    dloc = dst % SHARD
    tile = dloc // 128
    dst_local = (dloc % 128).astype(np.float32)

    key = ((core * NTILES + tile) * NGROUPS + grp).astype(np.int64)
    order = np.argsort(key, kind="stable")
    key_s = key[order]
    lidx_s = lidx[order]
    dstl_s = dst_local[order]
    counts = np.bincount(key_s, minlength=NCORES * NTILES * NGROUPS)
    counts = counts.reshape(NCORES, NTILES, NGROUPS)
    starts = np.zeros_like(counts)
    flat = counts.reshape(NCORES, -1)
    starts.reshape(NCORES, -1)[:, 1:] = np.cumsum(flat, axis=1)[:, :-1]
    starts += np.concatenate([[0], np.cumsum(flat.sum(axis=1))[:-1]]).reshape(-1, 1, 1)

    chunks_tg = -(-counts.max(axis=0) // 128)              # [NTILES, NGROUPS]

    supers = []
    t0 = 0
    for ssz in SUPER_SIZES:
        supers.append(list(range(t0, t0 + ssz)))
        t0 += ssz

    # slot layout: super-major, then group, then tile-in-super
    slot_off = np.zeros((NTILES, NGROUPS), np.int64)
    windows = []                       # per (super, group): (w0, nchunks)
    super_rng = []                     # per super: (slot0, slot1)
    pos = 0
    for tiles in supers:
        s0 = pos
        for g in range(NGROUPS):
            w0 = pos
            for t in tiles:
                slot_off[t, g] = pos
                pos += int(chunks_tg[t, g]) * 128
            windows.append((w0, (pos - w0) // 128))
        super_rng.append((s0, pos))
    S_total = pos
    C_total = S_total // 128

    idx_all = np.zeros((NCORES, S_total), np.int16)
    dst_all = np.full((NCORES, S_total), DST_SENTINEL, np.float32)
    for c in range(NCORES):
        for t in range(NTILES):
            for g in range(NGROUPS):
                n = int(counts[c, t, g])
                if n == 0:
                    continue
                s0 = int(starts[c, t, g])
                o = int(slot_off[t, g])
                idx_all[c, o:o + n] = lidx_s[s0:s0 + n]
                dst_all[c, o:o + n] = dstl_s[s0:s0 + n]

    idx_dev = np.ascontiguousarray(
        np.tile(idx_all.reshape(NCORES, -1, 16).transpose(0, 2, 1), (1, 8, 1)))
    dst_dev = np.ascontiguousarray(
        dst_all.reshape(NCORES, C_total, 128).transpose(0, 2, 1)).astype(ml_dtypes.bfloat16)

    x_pad = np.zeros((NCORES, PADSHARD, IN_FEAT), np.float32)
    x_pad[:, :SHARD] = x.reshape(NCORES, SHARD, IN_FEAT)
    # xT_dev[c, p, h, n] = x_pad[c, n, h*128+p]
    xT_dev = np.ascontiguousarray(
        x_pad.transpose(0, 2, 1).reshape(NCORES, 2, 128, PADSHARD)
        .transpose(0, 2, 1, 3)).astype(ml_dtypes.bfloat16)

    dinv_pad = np.zeros((NCORES, PADSHARD), np.float32)
    dinv_pad[:, :SHARD] = dinv.reshape(NCORES, SHARD)
    dinv_dev = np.ascontiguousarray(
        dinv_pad.reshape(NCORES, NTILES, 128).transpose(0, 2, 1))   # [C,128,98]

    iota_np = np.tile(np.arange(128, dtype=np.float32)[None, :], (128, 1)).astype(ml_dtypes.bfloat16)
    identb_np = np.eye(128, dtype=np.float32).astype(ml_dtypes.bfloat16)
    bias_np = np.tile(b_out[None, :], (128, 1)).astype(np.float32)
    W2_np = W2.astype(ml_dtypes.bfloat16)

    meta = dict(chunks_tg=chunks_tg, supers=supers, windows=windows,
                slot_off=slot_off, S_total=S_total, C_total=C_total,
                super_rng=super_rng)
    per_core = dict(xT=xT_dev, idx=idx_dev, dstv=dst_dev, dinv=dinv_dev)
    consts = dict(W2=W2_np, iota=iota_np, identb=identb_np, bias=bias_np)
    return meta, per_core, consts


def _build(meta):
    import concourse.bass as bass
    import concourse.tile as tile
    from concourse import bacc, mybir

    chunks_tg = meta["chunks_tg"]
    supers = meta["supers"]
    windows = meta["windows"]
    slot_off = meta["slot_off"]
    S_total = meta["S_total"]
    C_total = meta["C_total"]
    super_rng = meta["super_rng"]

    nc = bacc.Bacc("TRN2", target_bir_lowering=False, debug=False,
                   num_devices=NCORES, num_swdge_queues=4)
    f32, bf16, i16 = mybir.dt.float32, mybir.dt.bfloat16, mybir.dt.int16
    mul_op, add_op = mybir.AluOpType.mult, mybir.AluOpType.add

    xT_ap = nc.dram_tensor("xT_in", [128, 2, PADSHARD], bf16, kind="ExternalInput").ap()
    idx_ap = nc.dram_tensor("idx_in", [128, S_total // 16], i16, kind="ExternalInput").ap()
    dst_ap = nc.dram_tensor("dst_in", [128, C_total], bf16, kind="ExternalInput").ap()
    dinv_ap = nc.dram_tensor("dinv_in", [128, NTILES], f32, kind="ExternalInput").ap()
    W2_ap = nc.dram_tensor("w2_in", [IN_FEAT, OUT_FEAT], bf16, kind="ExternalInput").ap()
    iota_ap = nc.dram_tensor("iota_in", [128, 128], bf16, kind="ExternalInput").ap()
    identb_ap = nc.dram_tensor("identb_in", [128, 128], bf16, kind="ExternalInput").ap()
    bias_ap = nc.dram_tensor("bias_in", [128, OUT_FEAT], f32, kind="ExternalInput").ap()
    out_ap = nc.dram_tensor("y_out", [128, NTILES, OUT_FEAT], f32, kind="ExternalOutput").ap()

    with tile.TileContext(nc) as tc, ExitStack() as ctx:
        dram = ctx.enter_context(tc.tile_pool(name="dram", bufs=1, space="DRAM"))
        g_c = dram.tile([PADSHARD, OUT_FEAT], f32)
        g_full = dram.tile([GTAB_ROWS, OUT_FEAT], f32)

        cpool = ctx.enter_context(tc.tile_pool(name="consts", bufs=1))
        iota_t = cpool.tile([128, 128], bf16)
        nc.sync.dma_start(iota_t[:], iota_ap[:])
        identb_t = cpool.tile([128, 128], bf16)
        nc.sync.dma_start(identb_t[:], identb_ap[:])
        bias_t = cpool.tile([128, OUT_FEAT], f32)
        nc.sync.dma_start(bias_t[:], bias_ap[:])
        dinv_t = cpool.tile([128, NTILES], f32)
        nc.sync.dma_start(dinv_t[:], dinv_ap[:])
        dstv_t = cpool.tile([128, C_total], bf16)
        nc.sync.dma_start(dstv_t[:], dst_ap[:])
        w2_t = cpool.tile([128, 2, OUT_FEAT], bf16)
        nc.sync.dma_start(w2_t[:], W2_ap.rearrange("(k p) f -> p k f", p=128))
        gres = cpool.tile([128, NTILES, OUT_FEAT], bf16)

        # ---- phase 1: g'_c = (x_c @ W2) * dinv_c ----
        with tc.tile_pool(name="ph1", bufs=3) as ph1, \
             tc.tile_pool(name="ph1c", bufs=1) as ph1c, \
             tc.tile_pool(name="ph1ps", bufs=2, space="PSUM") as ph1ps:
            xT_t = ph1c.tile([128, 2, PADSHARD], bf16)
            nc.sync.dma_start(xT_t[:], xT_ap[:])
            for nt in range(NTILES):
                gp = ph1ps.tile([128, OUT_FEAT], f32, tag="gp")
                for h in range(2):
                    nc.tensor.matmul(gp[:], xT_t[:, h, nt * 128:(nt + 1) * 128],
                                     w2_t[:, h, :], start=(h == 0), stop=(h == 1))
                gs = ph1.tile([128, OUT_FEAT], f32, tag="gs")
                nc.scalar.mul(gs[:], gp[:], dinv_t[:, nt:nt + 1])
                nc.vector.tensor_copy(gres[:, nt, :], gs[:])
                nc.sync.dma_start(g_c[nt * 128:(nt + 1) * 128, :], gs[:])

        # ---- allgather g' ----
        nc.gpsimd.collective_compute(
            "AllGather", mybir.AluOpType.bypass,
            ins=[g_c.opt()], outs=[g_full.opt()],
            replica_groups=[list(range(NCORES))],
        )

        # ---- phase 2 ----
        idxp = ctx.enter_context(tc.tile_pool(name="idxp", bufs=2))
        p2 = ctx.enter_context(tc.tile_pool(name="p2", bufs=2))
        ohp = ctx.enter_context(tc.tile_pool(name="ohp", bufs=6))
        psum2 = ctx.enter_context(tc.tile_pool(name="ps2", bufs=1, space="PSUM"))
        outp = ctx.enter_context(tc.tile_pool(name="outp", bufs=2))

        win_i = 0
        qrr = 0
        for si, tiles in enumerate(supers):
            s0, s1 = super_rng[si]
            idx_t = idxp.tile([128, (s1 - s0) // 16], i16, tag="idx", bufs=3)
            nc.sync.dma_start(idx_t[:], idx_ap[:, s0 // 16:s1 // 16])

            accs = {}
            started = set()
            for ti in range(len(tiles)):
                accs[ti] = psum2.tile([128, OUT_FEAT], f32, tag=f"acc{ti}",
                                      name=f"acc_{si}_{ti}")

            for g in range(NGROUPS):
                w0, wch = windows[win_i]
                win_i += 1
                if wch == 0:
                    continue
                nsl = wch * 128
                msg32 = p2.tile([128, wch, OUT_FEAT], f32, tag=f"m32_{g}", bufs=2)
                nc.gpsimd.dma_gather(
                    msg32[:], g_full[g * GROUP_ROWS:(g + 1) * GROUP_ROWS, :],
                    idx_t[:, (w0 - s0) // 16:(w0 - s0 + nsl) // 16],
                    nsl, nsl, OUT_FEAT, single_packet=False, queue_num=qrr % 4,
                )
                qrr += 1
                msg16 = p2.tile([128, wch, OUT_FEAT], bf16, tag=f"m16_{g}", bufs=2)
                nc.scalar.copy(msg16[:], msg32[:])
                oh_win = ohp.tile([128, wch, 128], bf16, tag=f"oh_{g}", bufs=1)
                nc.vector.tensor_tensor(
                    out=oh_win[:],
                    in0=iota_t[:].unsqueeze(1).broadcast_to([128, wch, 128]),
                    in1=dstv_t[:, w0 // 128:w0 // 128 + wch]
                        .unsqueeze(2).broadcast_to([128, wch, 128]),
                    op=mybir.AluOpType.is_equal)
                for ti, t in enumerate(tiles):
                    ntch = int(chunks_tg[t, g])
                    if ntch == 0:
                        continue
                    c0 = (slot_off[t, g] - w0) // 128
                    for j in range(ntch):
                        nc.tensor.matmul(accs[ti][:], oh_win[:, c0 + j, :],
                                         msg16[:, c0 + j, :],
                                         start=(ti not in started), stop=False)
                        started.add(ti)

            stg = outp.tile([128, len(tiles), OUT_FEAT], f32, tag="stg")
            for ti, t in enumerate(tiles):
                nc.tensor.matmul(accs[ti][:], identb_t[:], gres[:, t, :],
                                 start=(ti not in started), stop=True)
                nc.vector.scalar_tensor_tensor(
                    out=stg[:, ti, :], in0=accs[ti][:],
                    scalar=dinv_t[:, t:t + 1], in1=bias_t[:],
                    op0=mul_op, op1=add_op)
            nc.sync.dma_start(out_ap[:, tiles[0]:tiles[0] + len(tiles), :], stg[:])

    nc.compile()
    return nc


_CACHED = {}


def kernel(x, edge_index, W_gc, b_gc, W_fc, b_fc):
    from concourse import bass_utils

    meta, per_core, consts = _preprocess(x, edge_index, W_gc, b_gc, W_fc, b_fc)
    cache_key = (meta["S_total"], meta["C_total"],
                 tuple(map(tuple, meta["chunks_tg"])))
    if cache_key in _CACHED:
        nc = _CACHED[cache_key]
    else:
        nc = _build(meta)
        _CACHED.clear()
        _CACHED[cache_key] = nc

    in_maps = []
    for c in range(NCORES):
        in_maps.append({
            "xT_in": per_core["xT"][c],
            "idx_in": per_core["idx"][c],
            "dst_in": per_core["dstv"][c],
            "dinv_in": per_core["dinv"][c],
            "w2_in": consts["W2"],
            "iota_in": consts["iota"],
            "identb_in": consts["identb"],
            "bias_in": consts["bias"],
        })
    res = bass_utils.run_bass_kernel_spmd(nc, in_maps, core_ids=list(range(NCORES)))
    out = np.empty((N_NODES, OUT_FEAT), np.float32)
    for c in range(NCORES):
        oc = res.results[c]["y_out"]
        out[c * SHARD:(c + 1) * SHARD] = (
            oc.transpose(1, 0, 2).reshape(PADSHARD, OUT_FEAT)[:SHARD])
    return out
